# revision 5
# baseline (speedup 1.0000x reference)
"""DynamicSequenceChunker TRN2 kernel.

Sharding: pure data-parallel over the batch dim (8 rows -> 8 NeuronCores),
W_qk / W_res / start_key replicated.

Per core, two SPMD device passes with light host index-prep between them:
  Pass A: tokens -> PE transpose -> fp32r QK matmul (transposed layout
          qkT[e,l]) -> shifted-k dot / norm reductions via ones-vector
          matmuls -> per-token d, |q|^2, |ks|^2.
  Host:   cos + exact fp64 re-check of sign-ambiguous tokens (fp32r's
          ~2^-12 element error vs the min |probs-0.5| margin), boundary
          bits, chunk cumsum, padded per-tile gather indices, gates.
  Pass B: tokens -> transpose -> fp32r residual matmul + b_res, gather ->
          tensor_tensor_scan (h = g*h + x) -> gather by chunk id, PE
          transpose-back accumulated into the residual PSUM, indirect-DMA
          row scatter for the compacted `downsampled` output.
"""
import numpy as np
import sys

sys.path.insert(0, "/opt/trn_rl_repo")

import concourse.bacc as bacc
import concourse.bass as bass
import concourse.mybir as mybir
from concourse.tile import TileContext
from concourse.bass_utils import run_bass_kernel_spmd

F32 = mybir.dt.float32
F32R = mybir.dt.float32r
I16 = mybir.dt.int16
I32 = mybir.dt.int32

B, L, D = 8, 4096, 1024
DQK = 1024
LT = 8          # l macro tiles
TL = L // LT    # 512 tokens per tile
NSUB = TL // 128  # 4
DT = D // 128   # 8 d tiles
KT = D // 128   # 8 contraction tiles
ET = 2 * DQK // 128  # 16 e tiles of qk
THRESH = 0.5
N_TARGET = 6.0
RATIO_W = 0.03
EPS = 1e-8
OOB_ROW = 1 << 20  # scatter index for padded slots (skipped via bounds_check)


def _load_weights_r(nc, cst, sb, w_dram, cols, tag):
    """DMA fp32 weights chunkwise, round to fp32r via ScalarE."""
    w_r = cst.tile([128, KT, cols], F32R, tag=tag)
    for kt in range(KT):
        for c0 in range(0, cols, 512):
            stg = sb.tile([128, 512], F32, tag=tag + "_stg")
            nc.gpsimd.dma_start(
                stg[:], w_dram[kt * 128:(kt + 1) * 128, c0:c0 + 512])
            nc.scalar.copy(w_r[:, kt, c0:c0 + 512], stg[:])
    return w_r


def _build_pass_a():
    nc = bacc.Bacc("TRN2", target_bir_lowering=False, debug=False, num_devices=B)
    tokens = nc.dram_tensor("tokens", [L, D], F32, kind="ExternalInput").ap()
    w_qk = nc.dram_tensor("w_qk", [D, 2 * DQK], F32, kind="ExternalInput").ap()
    skey = nc.dram_tensor("skey", [DQK], F32, kind="ExternalInput").ap()
    ident_in = nc.dram_tensor("ident", [128, 128], F32, kind="ExternalInput").ap()
    dots = nc.dram_tensor("dots", [3, L], F32, kind="ExternalOutput").ap()

    with TileContext(nc) as tc:
        with tc.tile_pool(name="cst", bufs=1) as cst, \
             tc.tile_pool(name="sb", bufs=1) as sb, \
             tc.tile_pool(name="stg", bufs=2) as stg_p, \
             tc.tile_pool(name="pst", bufs=2, space="PSUM") as pst, \
             tc.tile_pool(name="psq", bufs=3, space="PSUM") as psq, \
             tc.tile_pool(name="psd", bufs=1, space="PSUM") as psd:
            ident_f = cst.tile([128, 128], F32, tag="ident_f")
            nc.gpsimd.dma_start(ident_f[:], ident_in[:])
            ident = cst.tile([128, 128], F32, tag="ident")
            nc.scalar.copy(ident[:], ident_f[:])
            # warm PE<->ACT sync so later matmuls carry a single wait
            ps_w = pst.tile([128, TL], F32, tag="ps_t")
            nc.tensor.transpose(ps_w[:, 0:128], ident[:], ident[:])

            ones_f = cst.tile([128, 1], F32, tag="ones_f")
            nc.vector.memset(ones_f[:], 1.0)
            ones_r = cst.tile([128, 1], F32R, tag="ones_r")
            nc.scalar.copy(ones_r[:], ones_f[:])

            sk_f = cst.tile([128, 8], F32, tag="sk_f")
            nc.gpsimd.dma_start(sk_f[:], skey.rearrange("(et p) -> p et", p=128))
            sk_sb = cst.tile([128, 8], F32, tag="sk_sb")
            nc.scalar.copy(sk_sb[:], sk_f[:])

            w_r = _load_weights_r(nc, cst, stg_p, w_qk, 2 * DQK, "w")

            kbuf = cst.tile([128, 8, TL + 1], F32, tag="kbuf")
            k2buf = cst.tile([128, 8, TL + 1], F32R, tag="k2buf")

            for lt in range(LT):
                if lt == 0:
                    nc.scalar.copy(
                        kbuf[:, :, 0:1].rearrange("p a b -> p (a b)"), sk_sb[:])
                else:
                    nc.scalar.copy(kbuf[:, :, 0:1], kbuf[:, :, TL:TL + 1])
                nc.scalar.square(k2buf[:, :, 0:1], kbuf[:, :, 0:1])

                tok_nat = sb.tile([128, NSUB, D], F32, tag="tok_nat")
                nc.gpsimd.dma_start(
                    tok_nat[:],
                    tokens[lt * TL:(lt + 1) * TL, :].rearrange(
                        "(ls p) d -> p ls d", p=128),
                )
                tokT_r = sb.tile([128, DT, TL], F32R, tag="tokT_r")
                for dt in range(DT):
                    ps_t = pst.tile([128, TL], F32, tag="ps_t")
                    for ls in range(NSUB):
                        nc.tensor.transpose(
                            ps_t[:, ls * 128:(ls + 1) * 128],
                            tok_nat[:, ls, dt * 128:(dt + 1) * 128],
                            ident[:],
                        )
                    nc.scalar.copy(tokT_r[:, dt, :], ps_t[:])

                qT = sb.tile([128, 8, TL], F32, tag="qT")
                q2 = sb.tile([128, 8, TL], F32R, tag="q2")
                for et in range(ET):
                    ps_qk = psq.tile([128, TL], F32, tag="ps_qk")
                    for kt in range(KT):
                        nc.tensor.matmul(
                            ps_qk[:],
                            w_r[:, kt, et * 128:(et + 1) * 128],
                            tokT_r[:, kt, :],
                            start=(kt == 0),
                            stop=(kt == KT - 1),
                        )
                    if et < 8:
                        nc.scalar.copy(qT[:, et, :], ps_qk[:])
                        nc.scalar.square(q2[:, et, :], ps_qk[:])
                    else:
                        nc.scalar.copy(kbuf[:, et - 8, 1:TL + 1], ps_qk[:])
                        nc.scalar.square(k2buf[:, et - 8, 1:TL + 1], ps_qk[:])

                mT = sb.tile([128, 8, TL], F32R, tag="mT")
                nc.vector.tensor_mul(mT[:], qT[:], kbuf[:, :, 0:TL])

                for psname, src, row in (
                    ("ps_d", mT, 0), ("ps_qn", q2, 1), ("ps_kn", k2buf, 2)):
                    ps = psd.tile([1, TL], F32, tag=psname)
                    for et in range(8):
                        rhs = src[:, et, 0:TL]
                        nc.tensor.matmul(ps[:], ones_r[:], rhs,
                                         start=(et == 0), stop=(et == 7))
                    ev = stg_p.tile([1, TL], F32, tag=psname + "_ev")
                    nc.vector.tensor_copy(ev[:], ps[:])
                    nc.gpsimd.dma_start(
                        dots[row:row + 1, lt * TL:(lt + 1) * TL], ev[:])
    nc.finalize()
    return nc


def _build_pass_b():
    nc = bacc.Bacc("TRN2", target_bir_lowering=False, debug=False, num_devices=B)
    tokens = nc.dram_tensor("tokens", [L, D], F32, kind="ExternalInput").ap()
    w_res = nc.dram_tensor("w_res", [D, D], F32, kind="ExternalInput").ap()
    b_res = nc.dram_tensor("b_res", [D], F32, kind="ExternalInput").ap()
    ident_in = nc.dram_tensor("ident", [128, 128], F32, kind="ExternalInput").ap()
    xg_idx = nc.dram_tensor("xg_idx", [LT, 128, TL // 16], I16,
                            kind="ExternalInput").ap()
    up_idx = nc.dram_tensor("up_idx", [LT, 128, TL // 16], I16,
                            kind="ExternalInput").ap()
    gates_in = nc.dram_tensor("gates", [LT, 128, TL + 1], F32,
                              kind="ExternalInput").ap()
    bp_in = nc.dram_tensor("bp", [LT, 128, TL], F32, kind="ExternalInput").ap()
    row_idx = nc.dram_tensor("row_idx", [LT, NSUB, 128], I32,
                             kind="ExternalInput").ap()
    down = nc.dram_tensor("down", [L, D], F32, kind="ExternalOutput").ap()
    ups = nc.dram_tensor("ups", [L, D], F32, kind="ExternalOutput").ap()

    with TileContext(nc) as tc:
        with tc.tile_pool(name="cst", bufs=1) as cst, \
             tc.tile_pool(name="sb", bufs=1) as sb, \
             tc.tile_pool(name="stg", bufs=2) as stg_p, \
             tc.tile_pool(name="pst", bufs=2, space="PSUM") as pst, \
             tc.tile_pool(name="psu", bufs=2, space="PSUM") as psu, \
             tc.tile_pool(name="psd", bufs=2, space="PSUM") as psd:
            ident_f = cst.tile([128, 128], F32, tag="ident_f")
            nc.gpsimd.dma_start(ident_f[:], ident_in[:])
            ident = cst.tile([128, 128], F32, tag="ident")
            nc.scalar.copy(ident[:], ident_f[:])
            ps_w = pst.tile([128, TL], F32, tag="ps_t")
            nc.tensor.transpose(ps_w[:, 0:128], ident[:], ident[:])

            ones_f = cst.tile([1, 128], F32, tag="ones_f")
            nc.vector.memset(ones_f[:], 1.0)
            ones_r = cst.tile([1, 128], F32R, tag="ones_r")
            nc.scalar.copy(ones_r[:], ones_f[:])

            br_f = cst.tile([1, D], F32, tag="br_f")
            nc.gpsimd.dma_start(br_f[:], b_res.rearrange("(a d) -> a d", a=1))
            br_r = cst.tile([1, D], F32R, tag="br_r")
            nc.scalar.copy(br_r[:], br_f[:])

            w_r = _load_weights_r(nc, cst, stg_p, w_res, D, "w")

            carry = cst.tile([128, DT, 1], F32, tag="carry")
            nc.vector.memset(carry[:], 0.0)

            for lt in range(LT):
                tok_nat = sb.tile([128, NSUB, D], F32, tag="tok_nat")
                nc.gpsimd.dma_start(
                    tok_nat[:],
                    tokens[lt * TL:(lt + 1) * TL, :].rearrange(
                        "(ls p) d -> p ls d", p=128),
                )
                tokT_r = sb.tile([128, DT, TL], F32R, tag="tokT_r")
                for dt in range(DT):
                    ps_t = pst.tile([128, TL], F32, tag="ps_t")
                    for ls in range(NSUB):
                        nc.tensor.transpose(
                            ps_t[:, ls * 128:(ls + 1) * 128],
                            tok_nat[:, ls, dt * 128:(dt + 1) * 128],
                            ident[:],
                        )
                    nc.scalar.copy(tokT_r[:, dt, :], ps_t[:])

                xg_i = sb.tile([128, TL // 16], I16, tag="xg_i")
                nc.gpsimd.dma_start(xg_i[:], xg_idx[lt])
                up_i = sb.tile([128, TL // 16], I16, tag="up_i")
                nc.gpsimd.dma_start(up_i[:], up_idx[lt])
                gat_d = sb.tile([128, TL + 1], F32, tag="gat_d")
                nc.gpsimd.dma_start(gat_d[:], gates_in[lt])
                gat = sb.tile([128, TL + 1], F32, tag="gat")
                nc.vector.tensor_copy(gat[:], gat_d[:])
                bp_d = sb.tile([128, TL], F32, tag="bp_d")
                nc.gpsimd.dma_start(bp_d[:], bp_in[lt])
                bp = sb.tile([128, TL], F32, tag="bp")
                nc.vector.tensor_copy(bp[:], bp_d[:])

                # gather boundary-token columns; x = gathered * bprobs
                xg = sb.tile([128, DT, TL], F32, tag="xg")
                for dt in range(DT):
                    nc.gpsimd.ap_gather(
                        xg[:, dt, :], tokT_r[:, dt, :].bitcast(F32), xg_i[:],
                        channels=128, num_elems=TL, d=1, num_idxs=TL,
                    )
                x_seg = sb.tile([128, DT, TL + 1], F32, tag="x_seg")
                nc.vector.memset(x_seg[:, :, 0:1], 0.0)
                for dt in range(DT):
                    nc.vector.tensor_mul(x_seg[:, dt, 1:TL + 1], xg[:, dt, :],
                                         bp[:])

                scanned = sb.tile([128, DT, TL + 1], F32, tag="scanned")
                for dt in range(DT):
                    nc.vector.tensor_tensor_scan(
                        scanned[:, dt, :], gat[:], x_seg[:, dt, :],
                        carry[:, dt, :],
                        op0=mybir.AluOpType.mult, op1=mybir.AluOpType.add,
                    )
                nc.vector.tensor_copy(carry[:], scanned[:, :, TL:TL + 1])

                upsT = sb.tile([128, DT, TL], F32, tag="upsT")
                for dt in range(DT):
                    nc.gpsimd.ap_gather(
                        upsT[:, dt, :], scanned[:, dt, :], up_i[:],
                        channels=128, num_elems=TL + 1, d=1, num_idxs=TL,
                    )

                # down: transpose x_seg real columns back to [slot, d] + scatter
                for ls in range(NSUB):
                    dn_sb = sb.tile([128, D], F32, tag="dn_sb")
                    for half in range(2):
                        ps_dn = psd.tile([128, 512], F32, tag="ps_dn")
                        for j in range(4):
                            dt = half * 4 + j
                            nc.tensor.transpose(
                                ps_dn[:, j * 128:(j + 1) * 128],
                                x_seg[:, dt, 1 + ls * 128:1 + (ls + 1) * 128],
                                ident[:],
                            )
                        nc.vector.tensor_copy(
                            dn_sb[:, half * 512:(half + 1) * 512], ps_dn[:])
                    ridx = sb.tile([128, 1], I32, tag="ridx")
                    nc.gpsimd.dma_start(
                        ridx[:],
                        row_idx[lt, ls].rearrange("(p a) -> p a", a=1))
                    nc.gpsimd.indirect_dma_start(
                        out=down[:],
                        out_offset=bass.IndirectOffsetOnAxis(
                            ap=ridx[:, 0:1], axis=0),
                        in_=dn_sb[:],
                        in_offset=None,
                        bounds_check=L - 1,
                        oob_is_err=False,
                    )

                # ups: residual matmul + b_res + transposed scan-gather
                for ls in range(NSUB):
                    up_sb = sb.tile([128, D], F32, tag="up_sb")
                    for half in range(2):
                        ps_u = psu.tile([128, 512], F32, tag="ps_u")
                        for kt in range(KT):
                            nc.tensor.matmul(
                                ps_u[:],
                                tokT_r[:, kt, ls * 128:(ls + 1) * 128],
                                w_r[:, kt, half * 512:(half + 1) * 512],
                                start=(kt == 0), stop=False,
                            )
                        nc.tensor.matmul(
                            ps_u[:], ones_r[:],
                            br_r[:, half * 512:(half + 1) * 512],
                            start=False, stop=False,
                        )
                        for j in range(4):
                            dt = half * 4 + j
                            nc.tensor.matmul(
                                ps_u[:, j * 128:(j + 1) * 128],
                                upsT[:, dt, ls * 128:(ls + 1) * 128],
                                ident[:],
                                is_transpose=True,
                                start=False, stop=(j == 3),
                            )
                        nc.scalar.copy(up_sb[:, half * 512:(half + 1) * 512],
                                       ps_u[:])
                    nc.gpsimd.dma_start(
                        ups[lt * TL + ls * 128:lt * TL + (ls + 1) * 128, :],
                        up_sb[:],
                    )
    nc.finalize()
    return nc


_NC_A = None
_NC_B = None
_LAST_MAPS_A = None
_LAST_MAPS_B = None


def _host_prep(d, qn2, kn2, tokens, w_qk, skey):
    """Replicate the reference's boundary logic; fp64 fixup for ambiguous cos."""
    qn = np.sqrt(qn2.astype(np.float32))
    kn = np.sqrt(kn2.astype(np.float32))
    cos = (d / np.maximum(qn * kn, EPS)).astype(np.float32)

    ambig = np.abs(cos) < 2e-4
    ambig[0] = False
    idxs = np.nonzero(ambig)[0]
    if idxs.size:
        t64 = tokens.astype(np.float64)
        wq = w_qk[:, :DQK].astype(np.float64)
        wk = w_qk[:, DQK:].astype(np.float64)
        q = t64[idxs] @ wq
        ks = t64[np.maximum(idxs - 1, 0)] @ wk
        c = np.einsum('ij,ij->i', q, ks) / np.maximum(
            np.linalg.norm(q, axis=1) * np.linalg.norm(ks, axis=1), EPS)
        cos[idxs] = c.astype(np.float32)

    probs = ((np.float32(1.0) - cos) * np.float32(0.5)).astype(np.float32)
    boundary = probs > THRESH
    boundary[0] = True
    chunk_id = np.cumsum(boundary.astype(np.int64)) - 1

    xg_idx = np.zeros((LT, TL), np.int16)
    up_idx = np.zeros((LT, TL), np.int16)
    gates = np.ones((LT, TL + 1), np.float32)
    bp = np.zeros((LT, TL), np.float32)
    row_idx = np.full((LT, TL), OOB_ROW, np.int64)
    for lt in range(LT):
        lo, hi = lt * TL, (lt + 1) * TL
        b_loc = boundary[lo:hi]
        l_loc = np.nonzero(b_loc)[0]
        n = l_loc.size
        xg_idx[lt, :n] = l_loc
        gates[lt, 1:n + 1] = np.float32(1.0) - probs[lo:hi][l_loc]
        bp[lt, :n] = probs[lo:hi][l_loc]
        up_idx[lt] = np.cumsum(b_loc.astype(np.int64))
        start = chunk_id[lo] + (0 if b_loc[0] else 1)
        row_idx[lt, :n] = start + np.arange(n)

    def wrap(a):  # [LT, TL] -> [LT, 128, TL//16] wrapped per 16, replicated x8
        w = a.reshape(LT, TL // 16, 16).transpose(0, 2, 1)
        return np.ascontiguousarray(np.tile(w, (1, 8, 1)))

    F = boundary.astype(np.float32).mean(dtype=np.float32)
    G = probs.mean(dtype=np.float32)
    aux = (N_TARGET / (N_TARGET - 1.0)) * (
        (N_TARGET - 1.0) * F * G + (1.0 - F) * (1.0 - G))
    return dict(
        xg_idx=wrap(xg_idx).astype(np.int16),
        up_idx=wrap(up_idx).astype(np.int16),
        gates=np.ascontiguousarray(
            np.tile(gates[:, None, :], (1, 128, 1))).astype(np.float32),
        bp=np.ascontiguousarray(
            np.tile(bp[:, None, :], (1, 128, 1))).astype(np.float32),
        row_idx=np.minimum(row_idx, OOB_ROW).astype(np.int32).reshape(
            LT, NSUB, 128),
        aux=np.float32(aux),
    )


def kernel(tokens, W_qk, start_key, W_res, b_res):
    global _NC_A, _NC_B
    tokens = np.ascontiguousarray(np.asarray(tokens, np.float32))
    W_qk = np.ascontiguousarray(np.asarray(W_qk, np.float32))
    start_key = np.ascontiguousarray(np.asarray(start_key, np.float32))
    W_res = np.ascontiguousarray(np.asarray(W_res, np.float32))
    b_res = np.ascontiguousarray(np.asarray(b_res, np.float32))
    ident = np.eye(128, dtype=np.float32)

    if _NC_A is None:
        _NC_A = _build_pass_a()
    in_maps = [
        {"tokens": tokens[b], "w_qk": W_qk, "skey": start_key, "ident": ident}
        for b in range(B)
    ]
    global _LAST_MAPS_A
    _LAST_MAPS_A = in_maps
    res_a = run_bass_kernel_spmd(_NC_A, in_maps, list(range(B))).results

    preps = []
    for b in range(B):
        dots = res_a[b]["dots"]
        preps.append(_host_prep(dots[0], dots[1], dots[2], tokens[b], W_qk,
                                start_key))

    if _NC_B is None:
        _NC_B = _build_pass_b()
    in_maps_b = [
        {
            "tokens": tokens[b], "w_res": W_res, "b_res": b_res, "ident": ident,
            "xg_idx": preps[b]["xg_idx"], "up_idx": preps[b]["up_idx"],
            "gates": preps[b]["gates"], "bp": preps[b]["bp"],
            "row_idx": preps[b]["row_idx"],
        }
        for b in range(B)
    ]
    global _LAST_MAPS_B
    _LAST_MAPS_B = in_maps_b
    res_b = run_bass_kernel_spmd(_NC_B, in_maps_b, list(range(B))).results

    downsampled = np.stack([res_b[b]["down"] for b in range(B)])
    ups = np.stack([res_b[b]["ups"] for b in range(B)])
    weighted_aux = np.float32(np.mean([p["aux"] for p in preps]) * RATIO_W)
    return downsampled, ups, weighted_aux
